# revision 28
# baseline (speedup 1.0000x reference)
"""Bahdanau attention on 8 Trainium2 NeuronCores.

Reference computation (H=K=Q=V=2048, S=8192):
    pq = query @ Wq.T                      # [1,1,H]
    pk = key @ Wk.T                        # [1,S,H]
    scores = tanh(pq + pk) @ Ws.T          # [1,S,1] -> [1,1,S]
    attn = softmax(scores)                 # [1,1,S]
    context = attn @ values                # [1,1,V]
    returns (attn, context)

Sharding: sequence-parallel. Core c owns source positions
[c*1024, (c+1)*1024). Each core computes e_c[s] = exp(score[s]) for its
positions plus the partial context sum_c[s] e_c[s] * values[s, :].
tanh-scores are bounded (|score| <= ||Ws||_1), so exp without
max-subtraction is safe in fp32 and the cores need no communication;
the softmax normalizer Z = sum(e) is applied at the host gather step.

Per-core device program (all SPMD-identical, different data):
  - DMA priority order: keyT + first Wk half, then Wq (pq inputs), then
    the second Wk half, then values.  values half-tiles reuse the Wk
    SBUF slots (same pool tags) once the wk tiles served their last
    matmul, so values cost no extra SBUF.
  - pq = Wq @ q is computed off the critical path on the otherwise-idle
    ACT + DVE engines: ACT scales each WqT row-block by the matching
    query element (per-partition scale), DVE ping-pong-adds the 16
    partials, and 16 tiny PE matmuls (acc_slice^T @ ones) collapse the
    partition dim straight into the transposed [128, 16] bias layout.
  - pk^T[h, s] = WkT^T @ keyT in fp16, two 512-wide s-chunks per h-chunk
    accumulating in separate PSUM banks with their k-chunks interleaved
    (one stationary load feeds 1024 stream columns).  Accumulation
    groups never interleave within one PSUM bank (hardware start=True
    clears the whole bank); cross-bank interleaving is safe.
  - finished pk banks are staged to SBUF as fp16 by DVE so the PSUM
    slots recycle without waiting for tanh (tanh waits for pq, which
    lands only after the wq DMA stream).
  - t = tanh(pk + pq) fused on ACT (bias = pq column), float32r out;
    score[s] += Ws_m^T @ t_m accumulates over the 16 h-chunks in
    long-lived PSUM banks (float32r matmuls run at full PE speed).
  - e = exp(score) on ACT (fp16 first -- it gates the context matvec --
    then fp32 for the e_out output); e is transposed [1,1024]->[128,8]
    via 8 K=1 matmuls against a ones scalar.
  - ctx[v] += e_j^T @ values_j over the 8 s-chunks (fp16).
"""

import contextlib
import sys

for _p in ("/opt/trn_rl_repo", "/root/.axon_site"):
    if _p not in sys.path:
        sys.path.insert(0, _p)

import numpy as np

import concourse.bass as bass
import concourse.tile as tile
from concourse import bacc, mybir
from concourse.bass_utils import run_bass_kernel_spmd

N_CORES = 8
S = 8192
H = 2048
K = 2048
Q = 2048
V = 2048
SC = S // N_CORES          # 1024 source positions per core
KJ = K // 128              # 16 contraction chunks
HM = H // 128              # 16 hidden chunks
SN = SC // 512             # 2 moving-dim chunks of 512
SJ = SC // 128             # 8 s-chunks for the context matvec
VN = V // 512              # 4 context output chunks

F16 = mybir.dt.float16
F32 = mybir.dt.float32
F32R = mybir.dt.float32r
ACT = mybir.ActivationFunctionType


def _build_nc():
    nc = bacc.Bacc("TRN2", target_bir_lowering=False)

    keyT = nc.dram_tensor("keyT", [K, SC], F16, kind="ExternalInput")
    wkT = nc.dram_tensor("wkT", [K, H], F16, kind="ExternalInput")
    wqT = nc.dram_tensor("wqT", [Q, H], F16, kind="ExternalInput")
    qv = nc.dram_tensor("qv", [128, KJ], F32, kind="ExternalInput")
    ws = nc.dram_tensor("ws", [128, HM], F32R, kind="ExternalInput")
    values = nc.dram_tensor("values", [SC, V], F16, kind="ExternalInput")

    e_out = nc.dram_tensor("e_out", [1, SC], F32, kind="ExternalOutput")
    ctx_out = nc.dram_tensor("ctx_out", [1, V], F32, kind="ExternalOutput")

    with tile.TileContext(nc) as tc:
        with (
            tc.tile_pool(name="small", bufs=1) as small,
            tc.tile_pool(name="wq", bufs=3) as wq_pool,
            tc.tile_pool(name="sj", bufs=3) as sj_pool,
            tc.tile_pool(name="acc", bufs=1) as acc_pool,
            tc.tile_pool(name="kt", bufs=1) as kt_pool,
            tc.tile_pool(name="wk", bufs=1) as wk_pool,
            tc.tile_pool(name="pkc", bufs=24) as pkc_pool,
            tc.tile_pool(name="t", bufs=3) as t_pool,
            tc.tile_pool(name="auxps", bufs=2, space="PSUM") as aux_pool,
            tc.tile_pool(name="pkps", bufs=4, space="PSUM") as pk_psum,
            tc.tile_pool(name="scps", bufs=1, space="PSUM") as sc_psum,
        ):
            # ---- tiny constant loads -------------------------------------
            qv_sb = small.tile([128, KJ], F32, tag="qv")
            nc.sync.dma_start(out=qv_sb, in_=qv[:])
            ws_sb = small.tile([128, HM], F32R, tag="ws")
            nc.sync.dma_start(out=ws_sb, in_=ws[:])
            ones_col = small.tile([128, 1], F16, tag="onesc")
            nc.vector.memset(ones_col, 1.0)

            # ---- resident loads: kt + first wk half lead the DMA queue ---
            kt_t = []
            wk_a = []
            wk_b = []
            for j in range(KJ):
                t1 = kt_pool.tile([128, SC], F16, tag=f"kt{j}", name=f"kt{j}")
                nc.sync.dma_start(out=t1, in_=keyT[j * 128 : (j + 1) * 128, :])
                kt_t.append(t1)
                t2 = wk_pool.tile([128, H // 2], F16, tag=f"wka{j}", name=f"wka{j}")
                nc.sync.dma_start(
                    out=t2, in_=wkT[j * 128 : (j + 1) * 128, 0 : H // 2]
                )
                wk_a.append(t2)
            def wk_slice(j, m):
                if m < HM // 2:
                    return wk_a[j][:, m * 128 : (m + 1) * 128]
                return wk_b[j][:, (m - HM // 2) * 128 : (m - HM // 2 + 1) * 128]

            # ---- pq pipeline on the otherwise-idle ACT + DVE engines ------
            # s_j[p, h] = WqT[j*128+p, h] * q[j*128+p]   (ACT per-partition scale)
            # acc[p, h] = sum_j s_j[p, h]                (DVE ping-pong adds)
            # pq[m*128+mh] = sum_p acc[p, m-slice]       (PE: acc_slice^T @ ones
            #                -> [128, 1] psum = transposed pq column, 16 tiny MMs)
            # wq DMAs are issued after kt/wk so they don't delay the pk start.
            acc_half = [None, None]
            for j in range(KJ):
                wq_t = wq_pool.tile([128, H], F16, tag="wq")
                nc.sync.dma_start(out=wq_t, in_=wqT[j * 128 : (j + 1) * 128, :])
                s_j = sj_pool.tile([128, H], F16, tag="sj")
                nc.scalar.activation(
                    out=s_j,
                    in_=wq_t,
                    func=ACT.Identity,
                    scale=qv_sb[:, j : j + 1],
                )
                h = j % 2  # two parallel half-chains halve the serial latency
                if acc_half[h] is None:
                    acc_half[h] = s_j
                else:
                    acc_new = acc_pool.tile(
                        [128, H], F16, tag=f"acc{h}{j % 4 // 2}", name=f"acc{j}"
                    )
                    nc.vector.tensor_add(out=acc_new, in0=acc_half[h], in1=s_j)
                    acc_half[h] = acc_new
            acc_cur = acc_pool.tile([128, H], F16, tag="accf", name="accf")
            nc.vector.tensor_add(out=acc_cur, in0=acc_half[0], in1=acc_half[1])
            pq_sb = small.tile([128, HM], F32, tag="pq")

            for mm in range(HM):
                tp_ps = aux_pool.tile(
                    [128, 1], F32, tag="aux", name=f"pqt{mm}"
                )
                nc.tensor.matmul(
                    tp_ps,
                    lhsT=acc_cur[:, mm * 128 : (mm + 1) * 128],
                    rhs=ones_col,
                    start=True,
                    stop=True,
                )
                nc.vector.tensor_copy(out=pq_sb[:, mm : mm + 1], in_=tp_ps)

            for j in range(KJ):
                t3 = wk_pool.tile([128, H // 2], F16, tag=f"wkb{j}", name=f"wkb{j}")
                nc.sync.dma_start(
                    out=t3, in_=wkT[j * 128 : (j + 1) * 128, H // 2 : H]
                )
                wk_b.append(t3)


            # ---- main loop ------------------------------------------------
            # pk groups accumulate in 2 PSUM banks with k-chunks interleaved
            # (one stationary load feeds 1024 stream columns); each finished
            # bank is immediately staged to SBUF as fp16 by DVE so the PSUM
            # slot recycles without waiting for tanh (tanh waits for pq,
            # which lands only after the wq stream).
            sc_ps = []
            for n in range(SN):
                sc_ps.append(
                    sc_psum.tile([1, 512], F32, tag=f"sc{n}", name=f"sc{n}")
                )
            for m in range(HM):
                pk0 = pk_psum.tile([128, 512], F32, tag="pk", name=f"pk0_{m}")
                pk1 = pk_psum.tile([128, 512], F32, tag="pk", name=f"pk1_{m}")
                pks = (pk0, pk1)

                def pk_mm(n, j):
                    nc.tensor.matmul(
                        pks[n],
                        lhsT=wk_slice(j, m),
                        rhs=kt_t[j][:, n * 512 : (n + 1) * 512],
                        start=(j == 0),
                        stop=(j == KJ - 1),
                        skip_group_check=True,
                    )

                for j in range(KJ - 2):
                    pk_mm(0, j)
                    pk_mm(1, j)
                pk_mm(0, KJ - 2)
                pk_mm(0, KJ - 1)
                pk_mm(1, KJ - 2)
                pk_mm(1, KJ - 1)

                for n in range(SN):
                    t_sb = t_pool.tile([128, 512], F32R, tag="t")
                    if m < HM - 2:
                        pkc = pkc_pool.tile([128, 512], F16, tag="pkc")
                        nc.vector.tensor_copy(out=pkc, in_=pks[n])
                        tanh_in = pkc
                    else:
                        # tanh is caught up by the last waves; skip the
                        # staging copy and read PSUM directly
                        tanh_in = pks[n]
                    nc.scalar.activation(
                        out=t_sb,
                        in_=tanh_in,
                        func=ACT.Tanh,
                        bias=pq_sb[:, m : m + 1],
                    )
                    nc.tensor.matmul(
                        sc_ps[n],
                        lhsT=ws_sb[:, m : m + 1],
                        rhs=t_sb,
                        start=(m == 0),
                        stop=(m == HM - 1),
                        skip_group_check=True,
                    )

            # ---- values loads: reuse the wk SBUF slots (same tags) -------
            # Tile defers each DMA until the wk tile sharing the slot has
            # served its last pk matmul.
            val_t = {}
            # 32 value half-tiles onto the 32 wk tags; wka tags free early
            # (after m=7), wkb late (after m=15)
            _val_tags = [f"wka{i}" for i in range(KJ)] + [f"wkb{i}" for i in range(KJ)]
            vi = 0
            for j in range(SJ):
                for half in range(2):
                    tag = _val_tags[vi]
                    vi += 1
                    vt = wk_pool.tile(
                        [128, V // 2], F16, tag=tag, name=f"val{j}_{half}"
                    )
                    nc.sync.dma_start(
                        out=vt,
                        in_=values[
                            j * 128 : (j + 1) * 128,
                            half * (V // 2) : (half + 1) * (V // 2),
                        ],
                    )
                    val_t[(j, half)] = vt

            # ---- exp + outputs -------------------------------------------
            e32_sb = small.tile([1, SC], F32, tag="e32")
            e16_sb = small.tile([1, SC], F16, tag="e16")
            for n in range(SN):
                nc.scalar.activation(
                    out=e16_sb[:, n * 512 : (n + 1) * 512],
                    in_=sc_ps[n],
                    func=ACT.Exp,
                )

            # transpose e16 [1, SC] -> [128, SJ] via K=1 matmuls
            ones16 = small.tile([1, 1], F16, tag="ones16")
            nc.vector.memset(ones16, 1.0)
            e_t = small.tile([128, SJ], F16, tag="et")
            for j in range(SJ):
                tp_ps = aux_pool.tile([128, 1], F32, tag="aux", name=f"et{j}")
                nc.tensor.matmul(
                    tp_ps,
                    lhsT=e16_sb[:, j * 128 : (j + 1) * 128],
                    rhs=ones16,
                    start=True,
                    stop=True,
                )
                nc.vector.tensor_copy(out=e_t[:, j : j + 1], in_=tp_ps)

            # fp32 exp for the e_out output (not on the ctx critical path)
            for n in range(SN):
                nc.scalar.activation(
                    out=e32_sb[:, n * 512 : (n + 1) * 512],
                    in_=sc_ps[n],
                    func=ACT.Exp,
                )
            nc.sync.dma_start(out=e_out[:], in_=e32_sb[:])

            # ---- context matvec ------------------------------------------
            ctx_sb = small.tile([1, V], F32, tag="ctxsb")
            for v in range(VN):
                c_ps = aux_pool.tile([1, 512], F32, tag="aux", name=f"cps{v}")
                for j in range(SJ):
                    nc.tensor.matmul(
                        c_ps,
                        lhsT=e_t[:, j : j + 1],
                        rhs=val_t[(j, v // 2)][
                            :, (v % 2) * 512 : (v % 2) * 512 + 512
                        ],
                        start=(j == 0),
                        stop=(j == SJ - 1),
                    )
                nc.vector.tensor_copy(
                    out=ctx_sb[:, v * 512 : (v + 1) * 512], in_=c_ps
                )
            nc.sync.dma_start(out=ctx_out[:], in_=ctx_sb[:])

    nc.compile()
    return nc


_NC = None


def _get_nc():
    global _NC
    if _NC is None:
        _NC = _build_nc()
    return _NC


def _prepare_in_maps(query, key, values, Wq, Wk, Ws):
    wkT = np.ascontiguousarray(Wk.T).astype(np.float16)
    wqT = np.ascontiguousarray(Wq.T).astype(np.float16)
    qv = np.ascontiguousarray(
        query.reshape(KJ, 128).T
    ).astype(np.float32)
    ws = np.ascontiguousarray(Ws.reshape(HM, 128).T).astype(np.float32)
    key2d = key.reshape(S, K)
    in_maps = []
    for c in range(N_CORES):
        sl = slice(c * SC, (c + 1) * SC)
        in_maps.append(
            {
                "keyT": np.ascontiguousarray(key2d[sl].T).astype(np.float16),
                "wkT": wkT,
                "wqT": wqT,
                "qv": qv,
                "ws": ws,
                "values": np.ascontiguousarray(values[sl]).astype(np.float16),
            }
        )
    return in_maps


def _combine(results):
    e = np.concatenate([results[c]["e_out"].reshape(-1) for c in range(N_CORES)])
    ctx = np.sum(
        np.stack([results[c]["ctx_out"].reshape(-1) for c in range(N_CORES)]),
        axis=0,
        dtype=np.float64,
    )
    z = np.sum(e, dtype=np.float64)
    attn = (e.astype(np.float64) / z).astype(np.float32).reshape(1, 1, S)
    context = (ctx / z).astype(np.float32).reshape(1, 1, V)
    return attn, context


def run(query, key, values, Wq, Wk, Ws, trace=False, tmpdir=None, retries=2):
    nc = _get_nc()
    in_maps = _prepare_in_maps(query, key, values, Wq, Wk, Ws)
    last_err = None
    for _ in range(retries + 1):
        try:
            res = run_bass_kernel_spmd(
                nc,
                in_maps,
                core_ids=list(range(N_CORES)),
                trace=trace,
                tmpdir=tmpdir,
            )
            return _combine(res.results) + (res,)
        except Exception as err:  # transient device-state failures
            last_err = err
    raise last_err


def kernel(query, key, values, Wq, Wk, Ws):
    attn, context, _ = run(query, key, values, Wq, Wk, Ws)
    return attn, context


# revision 29
# speedup vs baseline: 1.0151x; 1.0151x over previous
"""Bahdanau attention on 8 Trainium2 NeuronCores.

Reference computation (H=K=Q=V=2048, S=8192):
    pq = query @ Wq.T                      # [1,1,H]
    pk = key @ Wk.T                        # [1,S,H]
    scores = tanh(pq + pk) @ Ws.T          # [1,S,1] -> [1,1,S]
    attn = softmax(scores)                 # [1,1,S]
    context = attn @ values                # [1,1,V]
    returns (attn, context)

Sharding: sequence-parallel. Core c owns source positions
[c*1024, (c+1)*1024). Each core computes e_c[s] = exp(score[s]) for its
positions plus the partial context sum_c[s] e_c[s] * values[s, :].
tanh-scores are bounded (|score| <= ||Ws||_1), so exp without
max-subtraction is safe in fp32 and the cores need no communication;
the softmax normalizer Z = sum(e) is applied at the host gather step.

Per-core device program (all SPMD-identical, different data):
  - DMA priority order: keyT + first Wk half, then Wq (pq inputs), then
    the second Wk half, then values.  values half-tiles reuse the Wk
    SBUF slots (same pool tags) once the wk tiles served their last
    matmul, so values cost no extra SBUF.
  - pq = Wq @ q is computed off the critical path on the otherwise-idle
    ACT + DVE engines: ACT scales each WqT row-block by the matching
    query element (per-partition scale), DVE ping-pong-adds the 16
    partials, and 16 tiny PE matmuls (acc_slice^T @ ones) collapse the
    partition dim straight into the transposed [128, 16] bias layout.
  - pk^T[h, s] = WkT^T @ keyT in fp16, two 512-wide s-chunks per h-chunk
    accumulating in separate PSUM banks with their k-chunks interleaved
    (one stationary load feeds 1024 stream columns).  Accumulation
    groups never interleave within one PSUM bank (hardware start=True
    clears the whole bank); cross-bank interleaving is safe.
  - finished pk banks are staged to SBUF as fp16 by DVE so the PSUM
    slots recycle without waiting for tanh (tanh waits for pq, which
    lands only after the wq DMA stream).
  - t = tanh(pk + pq) fused on ACT (bias = pq column), float32r out;
    score[s] += Ws_m^T @ t_m accumulates over the 16 h-chunks in
    long-lived PSUM banks (float32r matmuls run at full PE speed).
  - e = exp(score) on ACT (fp16 first -- it gates the context matvec --
    then fp32 for the e_out output); e is transposed [1,1024]->[128,8]
    via 8 K=1 matmuls against a ones scalar.
  - ctx[v] += e_j^T @ values_j over the 8 s-chunks (fp16).
"""

import contextlib
import sys

for _p in ("/opt/trn_rl_repo", "/root/.axon_site"):
    if _p not in sys.path:
        sys.path.insert(0, _p)

import numpy as np

import concourse.bass as bass
import concourse.tile as tile
from concourse import bacc, mybir
from concourse.bass_utils import run_bass_kernel_spmd

N_CORES = 8
S = 8192
H = 2048
K = 2048
Q = 2048
V = 2048
SC = S // N_CORES          # 1024 source positions per core
KJ = K // 128              # 16 contraction chunks
HM = H // 128              # 16 hidden chunks
SN = SC // 512             # 2 moving-dim chunks of 512
SJ = SC // 128             # 8 s-chunks for the context matvec
VN = V // 512              # 4 context output chunks

F16 = mybir.dt.float16
F32 = mybir.dt.float32
F32R = mybir.dt.float32r
ACT = mybir.ActivationFunctionType


def _build_nc():
    nc = bacc.Bacc("TRN2", target_bir_lowering=False)

    keyT = nc.dram_tensor("keyT", [K, SC], F16, kind="ExternalInput")
    wkT = nc.dram_tensor("wkT", [K, H], F16, kind="ExternalInput")
    wqT = nc.dram_tensor("wqT", [Q, H], F16, kind="ExternalInput")
    qv = nc.dram_tensor("qv", [128, KJ], F32, kind="ExternalInput")
    ws = nc.dram_tensor("ws", [128, HM], F32R, kind="ExternalInput")
    values = nc.dram_tensor("values", [SC, V], F16, kind="ExternalInput")

    e_out = nc.dram_tensor("e_out", [1, SC], F32, kind="ExternalOutput")
    ctx_out = nc.dram_tensor("ctx_out", [1, V], F32, kind="ExternalOutput")

    with tile.TileContext(nc) as tc:
        with (
            tc.tile_pool(name="small", bufs=1) as small,
            tc.tile_pool(name="wq", bufs=3) as wq_pool,
            tc.tile_pool(name="sj", bufs=3) as sj_pool,
            tc.tile_pool(name="acc", bufs=1) as acc_pool,
            tc.tile_pool(name="kt", bufs=1) as kt_pool,
            tc.tile_pool(name="wk", bufs=1) as wk_pool,
            tc.tile_pool(name="pkc", bufs=24) as pkc_pool,
            tc.tile_pool(name="t", bufs=3) as t_pool,
            tc.tile_pool(name="auxps", bufs=2, space="PSUM") as aux_pool,
            tc.tile_pool(name="pkps", bufs=4, space="PSUM") as pk_psum,
            tc.tile_pool(name="scps", bufs=1, space="PSUM") as sc_psum,
        ):
            # ---- tiny constant loads -------------------------------------
            qv_sb = small.tile([128, KJ], F32, tag="qv")
            nc.sync.dma_start(out=qv_sb, in_=qv[:])
            ws_sb = small.tile([128, HM], F32R, tag="ws")
            nc.sync.dma_start(out=ws_sb, in_=ws[:])
            ones_col = small.tile([128, 1], F16, tag="onesc")
            nc.vector.memset(ones_col, 1.0)

            # ---- resident loads: kt + first wk half lead the DMA queue ---
            kt_t = []
            wk_a = []
            wk_b = []
            for j in range(KJ):
                t1 = kt_pool.tile([128, SC], F16, tag=f"kt{j}", name=f"kt{j}")
                nc.sync.dma_start(out=t1, in_=keyT[j * 128 : (j + 1) * 128, :])
                kt_t.append(t1)
                t2 = wk_pool.tile([128, H // 2], F16, tag=f"wka{j}", name=f"wka{j}")
                nc.sync.dma_start(
                    out=t2, in_=wkT[j * 128 : (j + 1) * 128, 0 : H // 2]
                )
                wk_a.append(t2)
            def wk_slice(j, m):
                if m < HM // 2:
                    return wk_a[j][:, m * 128 : (m + 1) * 128]
                return wk_b[j][:, (m - HM // 2) * 128 : (m - HM // 2 + 1) * 128]

            # ---- pq pipeline on the otherwise-idle ACT + DVE engines ------
            # s_j[p, h] = WqT[j*128+p, h] * q[j*128+p]   (ACT per-partition scale)
            # acc[p, h] = sum_j s_j[p, h]                (DVE ping-pong adds)
            # pq[m*128+mh] = sum_p acc[p, m-slice]       (PE: acc_slice^T @ ones
            #                -> [128, 1] psum = transposed pq column, 16 tiny MMs)
            # wq DMAs are issued after kt/wk so they don't delay the pk start.
            acc_cur = None
            for j in range(KJ):
                wq_t = wq_pool.tile([128, H], F16, tag="wq")
                nc.sync.dma_start(out=wq_t, in_=wqT[j * 128 : (j + 1) * 128, :])
                s_j = sj_pool.tile([128, H], F16, tag="sj")
                nc.scalar.activation(
                    out=s_j,
                    in_=wq_t,
                    func=ACT.Identity,
                    scale=qv_sb[:, j : j + 1],
                )
                if acc_cur is None:
                    acc_cur = s_j
                else:
                    acc_new = acc_pool.tile(
                        [128, H], F16, tag=f"acc{j % 2}", name=f"acc{j}"
                    )
                    nc.vector.tensor_add(out=acc_new, in0=acc_cur, in1=s_j)
                    acc_cur = acc_new
            pq_sb = small.tile([128, HM], F32, tag="pq")

            for mm in range(HM):
                tp_ps = aux_pool.tile(
                    [128, 1], F32, tag="aux", name=f"pqt{mm}"
                )
                nc.tensor.matmul(
                    tp_ps,
                    lhsT=acc_cur[:, mm * 128 : (mm + 1) * 128],
                    rhs=ones_col,
                    start=True,
                    stop=True,
                )
                nc.vector.tensor_copy(out=pq_sb[:, mm : mm + 1], in_=tp_ps)

            for j in range(KJ):
                t3 = wk_pool.tile([128, H // 2], F16, tag=f"wkb{j}", name=f"wkb{j}")
                nc.sync.dma_start(
                    out=t3, in_=wkT[j * 128 : (j + 1) * 128, H // 2 : H]
                )
                wk_b.append(t3)


            # ---- main loop ------------------------------------------------
            # pk groups accumulate in 2 PSUM banks with k-chunks interleaved
            # (one stationary load feeds 1024 stream columns); each finished
            # bank is immediately staged to SBUF as fp16 by DVE so the PSUM
            # slot recycles without waiting for tanh (tanh waits for pq,
            # which lands only after the wq stream).
            sc_ps = []
            for n in range(SN):
                sc_ps.append(
                    sc_psum.tile([1, 512], F32, tag=f"sc{n}", name=f"sc{n}")
                )
            for m in range(HM):
                pk0 = pk_psum.tile([128, 512], F32, tag="pk", name=f"pk0_{m}")
                pk1 = pk_psum.tile([128, 512], F32, tag="pk", name=f"pk1_{m}")
                pks = (pk0, pk1)

                def pk_mm(n, j):
                    nc.tensor.matmul(
                        pks[n],
                        lhsT=wk_slice(j, m),
                        rhs=kt_t[j][:, n * 512 : (n + 1) * 512],
                        start=(j == 0),
                        stop=(j == KJ - 1),
                        skip_group_check=True,
                    )

                for j in range(KJ - 2):
                    pk_mm(0, j)
                    pk_mm(1, j)
                pk_mm(0, KJ - 2)
                pk_mm(0, KJ - 1)
                pk_mm(1, KJ - 2)
                pk_mm(1, KJ - 1)

                for n in range(SN):
                    t_sb = t_pool.tile([128, 512], F32R, tag="t")
                    if m < HM - 2:
                        pkc = pkc_pool.tile([128, 512], F16, tag="pkc")
                        nc.vector.tensor_copy(out=pkc, in_=pks[n])
                        tanh_in = pkc
                    else:
                        # tanh is caught up by the last waves; skip the
                        # staging copy and read PSUM directly
                        tanh_in = pks[n]
                    nc.scalar.activation(
                        out=t_sb,
                        in_=tanh_in,
                        func=ACT.Tanh,
                        bias=pq_sb[:, m : m + 1],
                    )
                    nc.tensor.matmul(
                        sc_ps[n],
                        lhsT=ws_sb[:, m : m + 1],
                        rhs=t_sb,
                        start=(m == 0),
                        stop=(m == HM - 1),
                        skip_group_check=True,
                    )

            # ---- values loads: reuse the wk SBUF slots (same tags) -------
            # Tile defers each DMA until the wk tile sharing the slot has
            # served its last pk matmul.
            val_t = {}
            # 32 value half-tiles onto the 32 wk tags; wka tags free early
            # (after m=7), wkb late (after m=15)
            _val_tags = [f"wka{i}" for i in range(KJ)] + [f"wkb{i}" for i in range(KJ)]
            vi = 0
            for j in range(SJ):
                for half in range(2):
                    tag = _val_tags[vi]
                    vi += 1
                    vt = wk_pool.tile(
                        [128, V // 2], F16, tag=tag, name=f"val{j}_{half}"
                    )
                    nc.sync.dma_start(
                        out=vt,
                        in_=values[
                            j * 128 : (j + 1) * 128,
                            half * (V // 2) : (half + 1) * (V // 2),
                        ],
                    )
                    val_t[(j, half)] = vt

            # ---- exp + outputs -------------------------------------------
            e32_sb = small.tile([1, SC], F32, tag="e32")
            e16_sb = small.tile([1, SC], F16, tag="e16")
            for n in range(SN):
                nc.scalar.activation(
                    out=e16_sb[:, n * 512 : (n + 1) * 512],
                    in_=sc_ps[n],
                    func=ACT.Exp,
                )

            # transpose e16 [1, SC] -> [128, SJ] via K=1 matmuls
            ones16 = small.tile([1, 1], F16, tag="ones16")
            nc.vector.memset(ones16, 1.0)
            e_t = small.tile([128, SJ], F16, tag="et")
            for j in range(SJ):
                tp_ps = aux_pool.tile([128, 1], F32, tag="aux", name=f"et{j}")
                nc.tensor.matmul(
                    tp_ps,
                    lhsT=e16_sb[:, j * 128 : (j + 1) * 128],
                    rhs=ones16,
                    start=True,
                    stop=True,
                )
                nc.vector.tensor_copy(out=e_t[:, j : j + 1], in_=tp_ps)

            # fp32 exp for the e_out output (not on the ctx critical path)
            for n in range(SN):
                nc.scalar.activation(
                    out=e32_sb[:, n * 512 : (n + 1) * 512],
                    in_=sc_ps[n],
                    func=ACT.Exp,
                )
            nc.sync.dma_start(out=e_out[:], in_=e32_sb[:])

            # ---- context matvec ------------------------------------------
            ctx_sb = small.tile([1, V], F32, tag="ctxsb")
            for v in range(VN):
                c_ps = aux_pool.tile([1, 512], F32, tag="aux", name=f"cps{v}")
                for j in range(SJ):
                    nc.tensor.matmul(
                        c_ps,
                        lhsT=e_t[:, j : j + 1],
                        rhs=val_t[(j, v // 2)][
                            :, (v % 2) * 512 : (v % 2) * 512 + 512
                        ],
                        start=(j == 0),
                        stop=(j == SJ - 1),
                    )
                nc.vector.tensor_copy(
                    out=ctx_sb[:, v * 512 : (v + 1) * 512], in_=c_ps
                )
            nc.sync.dma_start(out=ctx_out[:], in_=ctx_sb[:])

    nc.compile()
    return nc


_NC = None


def _get_nc():
    global _NC
    if _NC is None:
        _NC = _build_nc()
    return _NC


def _prepare_in_maps(query, key, values, Wq, Wk, Ws):
    wkT = np.ascontiguousarray(Wk.T).astype(np.float16)
    wqT = np.ascontiguousarray(Wq.T).astype(np.float16)
    qv = np.ascontiguousarray(
        query.reshape(KJ, 128).T
    ).astype(np.float32)
    ws = np.ascontiguousarray(Ws.reshape(HM, 128).T).astype(np.float32)
    key2d = key.reshape(S, K)
    in_maps = []
    for c in range(N_CORES):
        sl = slice(c * SC, (c + 1) * SC)
        in_maps.append(
            {
                "keyT": np.ascontiguousarray(key2d[sl].T).astype(np.float16),
                "wkT": wkT,
                "wqT": wqT,
                "qv": qv,
                "ws": ws,
                "values": np.ascontiguousarray(values[sl]).astype(np.float16),
            }
        )
    return in_maps


def _combine(results):
    e = np.concatenate([results[c]["e_out"].reshape(-1) for c in range(N_CORES)])
    ctx = np.sum(
        np.stack([results[c]["ctx_out"].reshape(-1) for c in range(N_CORES)]),
        axis=0,
        dtype=np.float64,
    )
    z = np.sum(e, dtype=np.float64)
    attn = (e.astype(np.float64) / z).astype(np.float32).reshape(1, 1, S)
    context = (ctx / z).astype(np.float32).reshape(1, 1, V)
    return attn, context


def run(query, key, values, Wq, Wk, Ws, trace=False, tmpdir=None, retries=2):
    nc = _get_nc()
    in_maps = _prepare_in_maps(query, key, values, Wq, Wk, Ws)
    last_err = None
    for _ in range(retries + 1):
        try:
            res = run_bass_kernel_spmd(
                nc,
                in_maps,
                core_ids=list(range(N_CORES)),
                trace=trace,
                tmpdir=tmpdir,
            )
            return _combine(res.results) + (res,)
        except Exception as err:  # transient device-state failures
            last_err = err
    raise last_err


def kernel(query, key, values, Wq, Wk, Ws):
    attn, context, _ = run(query, key, values, Wq, Wk, Ws)
    return attn, context


# revision 31
# speedup vs baseline: 1.0382x; 1.0228x over previous
"""Bahdanau attention on 8 Trainium2 NeuronCores.

Reference computation (H=K=Q=V=2048, S=8192):
    pq = query @ Wq.T                      # [1,1,H]
    pk = key @ Wk.T                        # [1,S,H]
    scores = tanh(pq + pk) @ Ws.T          # [1,S,1] -> [1,1,S]
    attn = softmax(scores)                 # [1,1,S]
    context = attn @ values                # [1,1,V]
    returns (attn, context)

Sharding: sequence-parallel. Core c owns source positions
[c*1024, (c+1)*1024). Each core computes e_c[s] = exp(score[s]) for its
positions plus the partial context sum_c[s] e_c[s] * values[s, :].
tanh-scores are bounded (|score| <= ||Ws||_1), so exp without
max-subtraction is safe in fp32 and the cores need no communication;
the softmax normalizer Z = sum(e) is applied at the host gather step.

Per-core device program (all SPMD-identical, different data):
  - DMA priority order: keyT + first Wk half, then Wq (pq inputs), then
    the second Wk half, then values.  values half-tiles reuse the Wk
    SBUF slots (same pool tags) once the wk tiles served their last
    matmul, so values cost no extra SBUF.
  - pq = Wq @ q is computed off the critical path on the otherwise-idle
    ACT + DVE engines: ACT scales each WqT row-block by the matching
    query element (per-partition scale), DVE ping-pong-adds the 16
    partials, and 16 tiny PE matmuls (acc_slice^T @ ones) collapse the
    partition dim straight into the transposed [128, 16] bias layout.
  - pk^T[h, s] = WkT^T @ keyT in fp16, two 512-wide s-chunks per h-chunk
    accumulating in separate PSUM banks with their k-chunks interleaved
    (one stationary load feeds 1024 stream columns).  Accumulation
    groups never interleave within one PSUM bank (hardware start=True
    clears the whole bank); cross-bank interleaving is safe.
  - finished pk banks are staged to SBUF as fp16 by DVE so the PSUM
    slots recycle without waiting for tanh (tanh waits for pq, which
    lands only after the wq DMA stream).
  - t = tanh(pk + pq) fused on ACT (bias = pq column), float32r out;
    score[s] += Ws_m^T @ t_m accumulates over the 16 h-chunks in
    long-lived PSUM banks (float32r matmuls run at full PE speed).
  - e = exp(score) on ACT (fp16 first -- it gates the context matvec --
    then fp32 for the e_out output); e is transposed [1,1024]->[128,8]
    via 8 K=1 matmuls against a ones scalar.
  - ctx[v] += e_j^T @ values_j over the 8 s-chunks (fp16).
"""

import contextlib
import sys

for _p in ("/opt/trn_rl_repo", "/root/.axon_site"):
    if _p not in sys.path:
        sys.path.insert(0, _p)

import numpy as np

import concourse.bass as bass
import concourse.tile as tile
from concourse import bacc, mybir
from concourse.bass_utils import run_bass_kernel_spmd

N_CORES = 8
S = 8192
H = 2048
K = 2048
Q = 2048
V = 2048
SC = S // N_CORES          # 1024 source positions per core
KJ = K // 128              # 16 contraction chunks
HM = H // 128              # 16 hidden chunks
SN = SC // 512             # 2 moving-dim chunks of 512
SJ = SC // 128             # 8 s-chunks for the context matvec
VN = V // 512              # 4 context output chunks

F16 = mybir.dt.float16
F32 = mybir.dt.float32
F32R = mybir.dt.float32r
ACT = mybir.ActivationFunctionType


def _build_nc():
    nc = bacc.Bacc("TRN2", target_bir_lowering=False)

    keyT = nc.dram_tensor("keyT", [K, SC], F16, kind="ExternalInput")
    wkT = nc.dram_tensor("wkT", [K, H], F16, kind="ExternalInput")
    wqT = nc.dram_tensor("wqT", [Q, H], F16, kind="ExternalInput")
    qv = nc.dram_tensor("qv", [128, KJ], F32, kind="ExternalInput")
    ws = nc.dram_tensor("ws", [128, HM], F32R, kind="ExternalInput")
    values = nc.dram_tensor("values", [SC, V], F16, kind="ExternalInput")

    e_out = nc.dram_tensor("e_out", [1, SC], F32, kind="ExternalOutput")
    ctx_out = nc.dram_tensor("ctx_out", [1, V], F32, kind="ExternalOutput")

    with tile.TileContext(nc) as tc:
        with (
            tc.tile_pool(name="small", bufs=1) as small,
            tc.tile_pool(name="wq", bufs=3) as wq_pool,
            tc.tile_pool(name="sj", bufs=3) as sj_pool,
            tc.tile_pool(name="acc", bufs=1) as acc_pool,
            tc.tile_pool(name="kt", bufs=1) as kt_pool,
            tc.tile_pool(name="wk", bufs=1) as wk_pool,
            tc.tile_pool(name="pkc", bufs=12) as pkc_pool,
            tc.tile_pool(name="t", bufs=3) as t_pool,
            tc.tile_pool(name="auxps", bufs=2, space="PSUM") as aux_pool,
            tc.tile_pool(name="pkps", bufs=2, space="PSUM") as pk_psum,
            tc.tile_pool(name="scps", bufs=1, space="PSUM") as sc_psum,
        ):
            # ---- tiny constant loads -------------------------------------
            qv_sb = small.tile([128, KJ], F32, tag="qv")
            nc.sync.dma_start(out=qv_sb, in_=qv[:])
            ws_sb = small.tile([128, HM], F32R, tag="ws")
            nc.sync.dma_start(out=ws_sb, in_=ws[:])
            ones_col = small.tile([128, 1], F16, tag="onesc")
            nc.vector.memset(ones_col, 1.0)

            # ---- resident loads: kt + first wk half lead the DMA queue ---
            kt_t = []
            wk_a = []
            wk_b = []
            for j in range(KJ):
                t1 = kt_pool.tile([128, SC], F16, tag=f"kt{j}", name=f"kt{j}")
                nc.sync.dma_start(out=t1, in_=keyT[j * 128 : (j + 1) * 128, :])
                kt_t.append(t1)
                t2 = wk_pool.tile([128, H // 2], F16, tag=f"wka{j}", name=f"wka{j}")
                nc.sync.dma_start(
                    out=t2, in_=wkT[j * 128 : (j + 1) * 128, 0 : H // 2]
                )
                wk_a.append(t2)
            def wk_slice(j, m):
                if m < HM // 2:
                    return wk_a[j][:, m * 128 : (m + 1) * 128]
                return wk_b[j][:, (m - HM // 2) * 128 : (m - HM // 2 + 1) * 128]

            # ---- pq pipeline on the otherwise-idle ACT + DVE engines ------
            # s_j[p, h] = WqT[j*128+p, h] * q[j*128+p]   (ACT per-partition scale)
            # acc[p, h] = sum_j s_j[p, h]                (DVE ping-pong adds)
            # pq[m*128+mh] = sum_p acc[p, m-slice]       (PE: acc_slice^T @ ones
            #                -> [128, 1] psum = transposed pq column, 16 tiny MMs)
            # wq DMAs are issued after kt/wk so they don't delay the pk start.
            acc_cur = None
            for j in range(KJ):
                wq_t = wq_pool.tile([128, H], F16, tag="wq")
                nc.sync.dma_start(out=wq_t, in_=wqT[j * 128 : (j + 1) * 128, :])
                s_j = sj_pool.tile([128, H], F16, tag="sj")
                nc.scalar.activation(
                    out=s_j,
                    in_=wq_t,
                    func=ACT.Identity,
                    scale=qv_sb[:, j : j + 1],
                )
                if acc_cur is None:
                    acc_cur = s_j
                else:
                    acc_new = acc_pool.tile(
                        [128, H], F16, tag=f"acc{j % 2}", name=f"acc{j}"
                    )
                    nc.vector.tensor_add(out=acc_new, in0=acc_cur, in1=s_j)
                    acc_cur = acc_new
            pq_sb = small.tile([128, HM], F32, tag="pq")

            for mm in range(HM):
                tp_ps = aux_pool.tile(
                    [128, 1], F32, tag="aux", name=f"pqt{mm}"
                )
                nc.tensor.matmul(
                    tp_ps,
                    lhsT=acc_cur[:, mm * 128 : (mm + 1) * 128],
                    rhs=ones_col,
                    start=True,
                    stop=True,
                )
                nc.vector.tensor_copy(out=pq_sb[:, mm : mm + 1], in_=tp_ps)

            for j in range(KJ):
                t3 = wk_pool.tile([128, H // 2], F16, tag=f"wkb{j}", name=f"wkb{j}")
                nc.sync.dma_start(
                    out=t3, in_=wkT[j * 128 : (j + 1) * 128, H // 2 : H]
                )
                wk_b.append(t3)


            # ---- main loop ------------------------------------------------
            # pk groups accumulate in 2 PSUM banks with k-chunks interleaved
            # (one stationary load feeds 1024 stream columns); each finished
            # bank is immediately staged to SBUF as fp16 by DVE so the PSUM
            # slot recycles without waiting for tanh (tanh waits for pq,
            # which lands only after the wq stream).
            sc_ps = []
            for n in range(SN):
                sc_ps.append(
                    sc_psum.tile([1, 512], F32, tag=f"sc{n}", name=f"sc{n}")
                )
            for m in range(HM):
                # one [128, 1024] tile spans 2 PSUM banks; each 512-wide
                # s-chunk is its own accumulation group in its own bank
                pk_t = pk_psum.tile([128, SC], F32, tag="pk", name=f"pk_{m}")

                def pk_mm(n, j):
                    nc.tensor.matmul(
                        pk_t[:, n * 512 : (n + 1) * 512],
                        lhsT=wk_slice(j, m),
                        rhs=kt_t[j][:, n * 512 : (n + 1) * 512],
                        start=(j == 0),
                        stop=(j == KJ - 1),
                        skip_group_check=True,
                    )

                for j in range(KJ - 2):
                    pk_mm(0, j)
                    pk_mm(1, j)
                pk_mm(0, KJ - 2)
                pk_mm(0, KJ - 1)
                pk_mm(1, KJ - 2)
                pk_mm(1, KJ - 1)

                # single wide cast + tanh per wave (halves per-op overhead)
                t_sb = t_pool.tile([128, SC], F32R, tag="t")
                if m < HM - 2:
                    pkc = pkc_pool.tile([128, SC], F16, tag="pkc")
                    nc.vector.tensor_copy(out=pkc, in_=pk_t)
                    tanh_in = pkc
                else:
                    tanh_in = pk_t
                nc.scalar.activation(
                    out=t_sb,
                    in_=tanh_in,
                    func=ACT.Tanh,
                    bias=pq_sb[:, m : m + 1],
                )
                for n in range(SN):
                    nc.tensor.matmul(
                        sc_ps[n],
                        lhsT=ws_sb[:, m : m + 1],
                        rhs=t_sb[:, n * 512 : (n + 1) * 512],
                        start=(m == 0),
                        stop=(m == HM - 1),
                        skip_group_check=True,
                    )

            # ---- values loads: reuse the wk SBUF slots (same tags) -------
            # Tile defers each DMA until the wk tile sharing the slot has
            # served its last pk matmul.
            val_t = {}
            # 32 value half-tiles onto the 32 wk tags; wka tags free early
            # (after m=7), wkb late (after m=15)
            _val_tags = [f"wka{i}" for i in range(KJ)] + [f"wkb{i}" for i in range(KJ)]
            vi = 0
            for j in range(SJ):
                for half in range(2):
                    tag = _val_tags[vi]
                    vi += 1
                    vt = wk_pool.tile(
                        [128, V // 2], F16, tag=tag, name=f"val{j}_{half}"
                    )
                    nc.sync.dma_start(
                        out=vt,
                        in_=values[
                            j * 128 : (j + 1) * 128,
                            half * (V // 2) : (half + 1) * (V // 2),
                        ],
                    )
                    val_t[(j, half)] = vt

            # ---- exp + outputs -------------------------------------------
            e32_sb = small.tile([1, SC], F32, tag="e32")
            e16_sb = small.tile([1, SC], F16, tag="e16")
            for n in range(SN):
                nc.scalar.activation(
                    out=e16_sb[:, n * 512 : (n + 1) * 512],
                    in_=sc_ps[n],
                    func=ACT.Exp,
                )

            # transpose e16 [1, SC] -> [128, SJ] via K=1 matmuls
            ones16 = small.tile([1, 1], F16, tag="ones16")
            nc.vector.memset(ones16, 1.0)
            e_t = small.tile([128, SJ], F16, tag="et")
            for j in range(SJ):
                tp_ps = aux_pool.tile([128, 1], F32, tag="aux", name=f"et{j}")
                nc.tensor.matmul(
                    tp_ps,
                    lhsT=e16_sb[:, j * 128 : (j + 1) * 128],
                    rhs=ones16,
                    start=True,
                    stop=True,
                )
                nc.vector.tensor_copy(out=e_t[:, j : j + 1], in_=tp_ps)

            # fp32 exp for the e_out output (not on the ctx critical path)
            for n in range(SN):
                nc.scalar.activation(
                    out=e32_sb[:, n * 512 : (n + 1) * 512],
                    in_=sc_ps[n],
                    func=ACT.Exp,
                )
            nc.sync.dma_start(out=e_out[:], in_=e32_sb[:])

            # ---- context matvec ------------------------------------------
            ctx_sb = small.tile([1, V], F32, tag="ctxsb")
            for v in range(VN):
                c_ps = aux_pool.tile([1, 512], F32, tag="aux", name=f"cps{v}")
                for j in range(SJ):
                    nc.tensor.matmul(
                        c_ps,
                        lhsT=e_t[:, j : j + 1],
                        rhs=val_t[(j, v // 2)][
                            :, (v % 2) * 512 : (v % 2) * 512 + 512
                        ],
                        start=(j == 0),
                        stop=(j == SJ - 1),
                    )
                nc.vector.tensor_copy(
                    out=ctx_sb[:, v * 512 : (v + 1) * 512], in_=c_ps
                )
            nc.sync.dma_start(out=ctx_out[:], in_=ctx_sb[:])

    nc.compile()
    return nc


_NC = None


def _get_nc():
    global _NC
    if _NC is None:
        _NC = _build_nc()
    return _NC


def _prepare_in_maps(query, key, values, Wq, Wk, Ws):
    wkT = np.ascontiguousarray(Wk.T).astype(np.float16)
    wqT = np.ascontiguousarray(Wq.T).astype(np.float16)
    qv = np.ascontiguousarray(
        query.reshape(KJ, 128).T
    ).astype(np.float32)
    ws = np.ascontiguousarray(Ws.reshape(HM, 128).T).astype(np.float32)
    key2d = key.reshape(S, K)
    in_maps = []
    for c in range(N_CORES):
        sl = slice(c * SC, (c + 1) * SC)
        in_maps.append(
            {
                "keyT": np.ascontiguousarray(key2d[sl].T).astype(np.float16),
                "wkT": wkT,
                "wqT": wqT,
                "qv": qv,
                "ws": ws,
                "values": np.ascontiguousarray(values[sl]).astype(np.float16),
            }
        )
    return in_maps


def _combine(results):
    e = np.concatenate([results[c]["e_out"].reshape(-1) for c in range(N_CORES)])
    ctx = np.sum(
        np.stack([results[c]["ctx_out"].reshape(-1) for c in range(N_CORES)]),
        axis=0,
        dtype=np.float64,
    )
    z = np.sum(e, dtype=np.float64)
    attn = (e.astype(np.float64) / z).astype(np.float32).reshape(1, 1, S)
    context = (ctx / z).astype(np.float32).reshape(1, 1, V)
    return attn, context


def run(query, key, values, Wq, Wk, Ws, trace=False, tmpdir=None, retries=2):
    nc = _get_nc()
    in_maps = _prepare_in_maps(query, key, values, Wq, Wk, Ws)
    last_err = None
    for _ in range(retries + 1):
        try:
            res = run_bass_kernel_spmd(
                nc,
                in_maps,
                core_ids=list(range(N_CORES)),
                trace=trace,
                tmpdir=tmpdir,
            )
            return _combine(res.results) + (res,)
        except Exception as err:  # transient device-state failures
            last_err = err
    raise last_err


def kernel(query, key, values, Wq, Wk, Ws):
    attn, context, _ = run(query, key, values, Wq, Wk, Ws)
    return attn, context
